# revision 38
# baseline (speedup 1.0000x reference)
"""AttnBlock (GroupNorm + self-attn + cross-attn + proj, residual) on 8 trn2 cores.

Sharding: data-parallel over batch B=16 -> 2 images per core; weights replicated.

v6: v5 + critical-path surgery.
 - head: x DMA gets the rings first (weights trail on the same queue);
   affine(0) and gT(0) evacuations split ACT/DVE so the serial
   GN -> affine -> g -> exp chain crosses engines in parallel halves.
 - kc/vc biases ride tiny PE bias-matmuls (brow stationary rows) so the
   stage-B evacuations collapse to one instruction each; kc+vcT share
   one PSUM bank per image.
 - gv(1) is emitted after image 0's exp stream starts; image 1's whole
   GN/gv pipeline hides under image 0's 8 exps.
 - tail: c_fin split into crp/rcinv/escn (escn on the idle GPSIMD) and
   attnV/y/dma stages, interleaved across the four (b, nh) pairs so the
   four cross-attn chains pipeline instead of running serially.
 - PSUM: 3x2-bank pool for the wide matmuls + 2x1-bank pool (warmup,
   groupnorm selector/broadcast, kc/vc/vcp, sc, crossrowsum).

Scale ledger (host WS=16 on true weights):
  M2' = GMS*(Wq^T Wk), GMS=128; gT = GQS*g + GQS*(bq Wk), GQS=8
  S^T psum = GQS*logits -> exp(scale=1/(16*GQS))
  v' = VSC*(hn Wv^T), VSC=2 = ones_self -> rinv = 1/(VSC*r), tmp = U/r fp8
  qc psum = WS*(Wqc(hn+tmp)); qcT = QS*qc + QS*(bqc + Wqc bv_s), QS=2
  kc psum = WS*(kc+bkc) (bias matmul) -> kcT = QS*(kc+bkc)
  vc psum = WS*(vc+bvc) -> vc_f8 = VS2*vc'; vcp psum = VS2*WS*(vc Wp^T)+bp
  vcp bf16 natural; onesc = 1/HCS (HCS=8)
  rcinv = HCS/rc; escn = E*rcinv (gpsimd); hc psum = HCS*out; y = psum/HCS + x
"""

import os

import numpy as np

B, C, H, W, S, CD = 16, 256, 32, 32, 77, 512
HW = H * W
SP = 80  # S padded to a 16B-aligned stride for DoubleRow APs
GROUPS = 32
GS = C // GROUPS
EPS = 1e-5
NCORES = 8
BPC = B // NCORES

WS = 16.0          # host-side weight scale (fp8 subnormal avoidance)
QS = 2.0           # qc/kc storage scale
GMS = 128.0        # host scale on M2 = Wq^T Wk
GQS = 8.0          # gT storage scale
EXPS_S = 1.0 / (16.0 * GQS)    # self exp scale
EXPS_C = 1.0 / (16.0 * QS * QS)  # cross exp scale
VSC = 2.0          # v storage scale == ones_self value
VS2 = 4.0          # vc fp8 storage scale
HCS = 8.0          # ones_cross = 1/HCS; final evac scale 1/HCS

# packed weight layout: name -> (k0, kch) into wall [128, 16, 2, 128]
WPACK = {"m2": (0, 2), "wv_s": (2, 2), "wq_c": (4, 2), "w_proj": (6, 2),
         "wk_c": (8, 4), "wv_c": (12, 4)}
CPACK = ["gn_gamma", "gn_beta", "bg2", "bq_c2"]

_CACHE = {}
LAST_RESULT = None  # test harness reads exec_time_ns off this


def _build_nc():
    import concourse.bacc as bacc
    import concourse.bass as bass
    import concourse.tile as tile
    from concourse import mybir

    f32 = mybir.dt.float32
    bf16 = mybir.dt.bfloat16
    fp8 = mybir.dt.float8e4
    AF = mybir.ActivationFunctionType
    OP = mybir.AluOpType
    DR = mybir.MatmulPerfMode.DoubleRow

    nc = bacc.Bacc("TRN2", target_bir_lowering=False, debug=False)

    x_d = nc.dram_tensor("xbf", [BPC, 128, 2, HW], bf16, kind="ExternalInput")
    cembT_d = nc.dram_tensor("cembT", [BPC, 128, 4 * SP], fp8,
                             kind="ExternalInput")
    wall_d = nc.dram_tensor("wall", [128, 16, 2, 128], fp8,
                            kind="ExternalInput")
    cols_d = nc.dram_tensor("cols", [128, 2 * len(CPACK)], f32,
                            kind="ExternalInput")
    brow_d = nc.dram_tensor("brow", [3, C], bf16, kind="ExternalInput")
    ident_d = nc.dram_tensor("ident", [128, 128], bf16, kind="ExternalInput")
    gsel_d = nc.dram_tensor("gsel", [128, 16], f32, kind="ExternalInput")
    gbc_d = nc.dram_tensor("gbc", [16, 128], f32, kind="ExternalInput")
    y_d = nc.dram_tensor("y", [BPC, 128, 2, 2, 512], bf16,
                         kind="ExternalOutput")

    with tile.TileContext(nc) as tc:
        with (
            tc.tile_pool(name="const", bufs=1) as const,
            tc.tile_pool(name="work", bufs=2) as work,
            tc.tile_pool(name="psp", bufs=3, space="PSUM") as psp,
            tc.tile_pool(name="pss", bufs=2, space="PSUM") as pss,
        ):
            # ---- constants (no DMA) ----
            ones2 = const.tile([128, 2, 128], fp8)
            nc.vector.memset(ones2, VSC)
            onesc = const.tile([S, 128], bf16)
            nc.vector.memset(onesc, 1.0 / HCS)
            ones_row = const.tile([1, 512], bf16)
            nc.vector.memset(ones_row, 1.0)
            # touch Exp once so its ACT table load overlaps the weight DMAs
            warm = const.tile([128, 1], f32)
            nc.vector.memset(warm, 0.0)
            nc.scalar.activation(warm, warm, AF.Exp)
            dummy_mov = const.tile([128, 2, 512], fp8)
            nc.vector.memset(dummy_mov, 1.0)

            ps = lambda: psp.tile([128, HW], f32, tag="ps", name="ps")
            ps5 = lambda: pss.tile([128, 512], f32, tag="sm", name="sm")

            def warm_burst(n):
                wup = ps5()
                for i in range(n):
                    nc.tensor.matmul(wup, ones2, dummy_mov,
                                     start=(i == 0), stop=(i == n - 1),
                                     perf_mode=DR)

            warm_burst(10)

            # ---- input DMAs ----
            # sync queue: x first (GN head is the critical path), then the
            # packed weights; scalar queue: selectors + cemb + bias rows
            xTs, cembTs = [], []
            for b in range(BPC):
                xT = work.tile([128, 2, HW], bf16, tag="xT")
                for a in range(2):
                    nc.sync.dma_start(out=xT[:, a, :], in_=x_d[b][:, a, :])
                xTs.append(xT)
            wall = const.tile([128, 16, 2, 128], fp8)
            nc.sync.dma_start(out=wall, in_=wall_d[:])
            gsel = const.tile([128, 16], f32)
            nc.scalar.dma_start(out=gsel, in_=gsel_d[:])
            gbc = const.tile([16, 128], f32)
            nc.scalar.dma_start(out=gbc, in_=gbc_d[:])
            for b in range(BPC):
                cembT = work.tile([128, 4, SP], fp8, tag="cembT")
                nc.scalar.dma_start(
                    out=cembT,
                    in_=cembT_d[b].rearrange("p (k s) -> p k s", s=SP))
                cembTs.append(cembT)
            ident = const.tile([128, 128], bf16)
            nc.scalar.dma_start(out=ident, in_=ident_d[:])
            brows = []
            for i in range(3):
                r = const.tile([1, C], bf16, tag=f"brow{i}")
                nc.scalar.dma_start(out=r, in_=brow_d[i:i + 1, :])
                brows.append(r)
            # gpsimd queue: packed bias/affine columns
            cols_all = const.tile([128, 2 * len(CPACK)], f32)
            nc.gpsimd.dma_start(out=cols_all, in_=cols_d[:])
            cols = {name: cols_all[:, 2 * i:2 * i + 2]
                    for i, name in enumerate(CPACK)}

            def wsl(name):
                k0, kch = WPACK[name]
                return wall[:, k0:k0 + kch, :, :]

            wvs_flat = wsl("wv_s").rearrange("p k m c -> p k (m c)")
            wproj_flat = wsl("w_proj").rearrange("p k m c -> p k (m c)")

            nb = lambda ap, nh: ap[:, nh * 512:(nh + 1) * 512]

            # per-image tiles
            T = [dict(xT=xTs[b], cembT=cembTs[b]) for b in range(BPC)]
            for b in range(BPC):
                t = T[b]
                for key, shape, dt_ in [
                    ("kcT", [128, 2, SP], fp8),
                    ("vc_f8", [128, 2, SP], fp8),
                    ("vcp", [S, C], bf16),
                    ("stats6", [128, 2, 2, 6], f32),
                    ("qsum", [128, 2, 2], f32),
                    ("m2sum", [128, 2, 2], f32),
                    ("msq_e", [128, 2, 2], f32),
                    ("musq", [128, 2, 2], f32),
                    ("spack", [128, 3, 2, 1], f32),
                    ("tm", [16, 2], f32),
                    ("ex2", [16, 2], f32),
                    ("msq", [16, 2], f32),
                    ("varv", [16, 2], f32),
                    ("ya", [16, 2], f32),
                    ("yb", [16, 2], f32),
                    ("y2", [16, 2], f32),
                    ("mrp", [16, 4], f32),
                    ("Acol", [128, 2], f32),
                    ("Bcol", [128, 2], f32),
                    ("t1", [128, 2], f32),
                    ("hnmm", [128, 2, HW], fp8),
                    ("gT", [128, 2, HW], fp8),
                    ("v_nat", [128, 8, C], fp8),
                    ("expST", [128, 8, HW], fp8),
                    ("rinv", [128, HW], f32),
                    ("tmp", [128, 2, HW], fp8),
                    ("qcT", [128, 2, HW], fp8),
                    ("expScT", [S, HW], bf16),
                    ("escn", [S, HW], bf16),
                    ("rcinv", [128, HW], f32),
                    ("y_sb", [128, 2, HW], bf16),
                ]:
                    t[key] = work.tile(shape, dt_, tag=key, name=key)

            def gn_stats(b):
                t = T[b]
                AX = mybir.AxisListType
                for a in range(2):
                    for ch in range(2):
                        nc.vector.bn_stats(
                            t["stats6"][:, a, ch, :],
                            t["xT"][:, a, ch * 512:(ch + 1) * 512])
                s6 = t["stats6"]
                m_e, m_o = s6[:, :, :, 1:2], s6[:, :, :, 4:5]
                v_e, v_o = s6[:, :, :, 2:3], s6[:, :, :, 5:6]
                nc.vector.tensor_add(t["qsum"], m_e, m_o)
                nc.vector.tensor_add(t["m2sum"], v_e, v_o)
                nc.vector.tensor_mul(t["msq_e"], m_e, m_e)
                nc.vector.tensor_mul(t["musq"], m_o, m_o)
                nc.vector.tensor_add(t["musq"], t["musq"], t["msq_e"])
                nc.vector.reduce_sum(out=t["spack"][:, 0, :, :],
                                     in_=t["qsum"], axis=AX.X)
                nc.vector.reduce_sum(out=t["spack"][:, 1, :, :],
                                     in_=t["m2sum"], axis=AX.X)
                nc.vector.reduce_sum(out=t["spack"][:, 2, :, :],
                                     in_=t["musq"], axis=AX.X)

            def gn_group(b):
                t = T[b]
                gps = ps5()
                nc.tensor.matmul(gps[0:16, 0:6], gsel, t["spack"],
                                 start=True, stop=True)
                nc.vector.tensor_scalar_mul(t["mrp"][:, 0:2], gps[0:16, 0:2],
                                            1.0 / 32.0)
                nc.vector.tensor_scalar_mul(t["tm"], gps[0:16, 2:4],
                                            1.0 / 8192.0)
                nc.vector.scalar_tensor_tensor(
                    out=t["ex2"], in0=gps[0:16, 4:6], scalar=1.0 / 32.0,
                    in1=t["tm"], op0=OP.mult, op1=OP.add)
                nc.vector.tensor_mul(t["msq"], t["mrp"][:, 0:2],
                                     t["mrp"][:, 0:2])
                nc.vector.tensor_sub(t["varv"], t["ex2"], t["msq"])
                nc.vector.tensor_scalar_add(t["varv"], t["varv"], EPS)
                nc.vector.reciprocal_approx_fast(out=t["ya"], in_=t["varv"])
                cur = t["ya"]
                for it in range(1):
                    nc.vector.tensor_mul(t["y2"], cur, cur)
                    nc.vector.tensor_mul(t["y2"], t["y2"], t["varv"])
                    nc.vector.tensor_scalar(out=t["y2"], in0=t["y2"],
                                            scalar1=-0.5, scalar2=1.5,
                                            op0=OP.mult, op1=OP.add)
                    nxt = t["yb"] if cur is t["ya"] else t["ya"]
                    nc.vector.tensor_mul(nxt, cur, t["y2"])
                    cur = nxt
                nc.vector.tensor_copy(t["mrp"][:, 2:4], cur)

            def gn_bcast(b):
                t = T[b]
                mps = ps5()
                nc.tensor.matmul(mps[0:128, 0:4], gbc, t["mrp"],
                                 start=True, stop=True)
                t["mps"] = mps

            def gn_affine(b):
                # image 0: a=0 on ACT, a=1 on DVE (parallel halves)
                t = T[b]
                mps = t["mps"]
                nc.vector.tensor_mul(t["Acol"], mps[0:128, 2:4],
                                     cols["gn_gamma"])
                nc.vector.tensor_mul(t["t1"], mps[0:128, 0:2], t["Acol"])
                nc.vector.tensor_sub(t["Bcol"], cols["gn_beta"], t["t1"])
                for a in range(2):
                    if b == 0 and a == 0:
                        nc.scalar.activation(
                            out=t["hnmm"][:, a, :], in_=t["xT"][:, a, :],
                            func=AF.Identity,
                            bias=t["Bcol"][:, a:a + 1],
                            scale=t["Acol"][:, a:a + 1])
                    else:
                        nc.vector.tensor_scalar(
                            out=t["hnmm"][:, a, :], in0=t["xT"][:, a, :],
                            scalar1=t["Acol"][:, a:a + 1],
                            scalar2=t["Bcol"][:, a:a + 1],
                            op0=OP.mult, op1=OP.add)

            def stageB_mm(b):
                # kc and vcT share one 1-bank psum; biases via tiny matmuls
                t = T[b]
                bg = ps5()
                t["bigB"] = bg
                for w, wname, brow_i in ((0, "wk_c", 1), (1, "wv_c", 2)):
                    for mc in range(2):
                        dst = bg[:, (2 * w + mc) * 128:(2 * w + mc) * 128 + SP]
                        nc.tensor.matmul(
                            dst, brows[brow_i][0:1,
                                               mc * 128:(mc + 1) * 128],
                            ones_row[0:1, 0:SP],
                            start=True, stop=False, skip_group_check=True)
                        for i in range(2):
                            nc.tensor.matmul(
                                dst, wsl(wname)[:, 2 * i:2 * i + 2, mc, :],
                                t["cembT"][:, 2 * i:2 * i + 2, :],
                                start=False, stop=(i == 1), perf_mode=DR,
                                skip_group_check=True)

            def stageB_evac(b):
                t = T[b]
                bg = t["bigB"]
                nc.vector.memset(t["kcT"][:, :, S:SP], 0.0)
                nc.vector.memset(t["vc_f8"][:, :, S:SP], 0.0)
                src = bg[:].rearrange("p (g s) -> p g s", s=128)
                nc.scalar.mul(t["kcT"][:, :, 0:S], src[:, 0:2, 0:S], QS / WS)
                nc.vector.tensor_scalar_mul(
                    t["vc_f8"][:, :, 0:S], src[:, 2:4, 0:S], VS2 / WS)

            def vcp_mm(b):
                # vcp = vc @ Wp^T + bp on [77, 256] (proj folded into values)
                t = T[b]
                vcp_ps = ps5()
                nc.tensor.matmul(
                    vcp_ps[0:SP, 0:C], ones_row[0:1, 0:SP], brows[0][0:1, :],
                    start=True, stop=False, skip_group_check=True)
                nc.tensor.matmul(
                    vcp_ps[0:SP, 0:C], t["vc_f8"][:, :, 0:SP], wproj_flat,
                    start=False, stop=True, perf_mode=DR,
                    skip_group_check=True)
                nc.vector.tensor_scalar_mul(
                    t["vcp"], vcp_ps[0:S, 0:C], 1.0 / (VS2 * WS))

            def gv_g(b):
                # g = hn M2 + bg; image 0 evacs split ACT/DVE
                t = T[b]
                for mc in range(2):
                    qp = ps()
                    for nh in range(2):
                        nc.tensor.matmul(
                            nb(qp, nh), wsl("m2")[:, :, mc, :],
                            t["hnmm"][:, :, nh * 512:(nh + 1) * 512],
                            start=True, stop=True, perf_mode=DR)
                    if b == 0 and mc == 0:
                        nc.scalar.activation(
                            out=t["gT"][:, mc, :], in_=qp, func=AF.Identity,
                            bias=cols["bg2"][:, mc:mc + 1], scale=GQS / GMS)
                    else:
                        nc.vector.tensor_scalar(
                            out=t["gT"][:, mc, :], in0=qp, scalar1=GQS / GMS,
                            scalar2=cols["bg2"][:, mc:mc + 1],
                            op0=OP.mult, op1=OP.add)

            def gv_v(b):
                t = T[b]
                for half in range(2):
                    vp = ps()
                    for j in range(4):
                        m8 = 4 * half + j
                        nc.tensor.matmul(
                            vp[:, j * 256:(j + 1) * 256],
                            t["hnmm"][:, :, m8 * 128:(m8 + 1) * 128],
                            wvs_flat,
                            start=True, stop=True, perf_mode=DR)
                    vdst = t["v_nat"][:, 4 * half:4 * half + 4, :]
                    vsrc = vp[:].rearrange("p (j c) -> p j c", c=256)
                    nc.vector.tensor_scalar_mul(vdst, vsrc, VSC / WS)

            def spexp_one(b, m8):
                t = T[b]
                sp = ps()
                for nh in range(2):
                    nc.tensor.matmul(
                        nb(sp, nh), t["hnmm"][:, :, m8 * 128:(m8 + 1) * 128],
                        t["gT"][:, :, nh * 512:(nh + 1) * 512],
                        start=True, stop=True, perf_mode=DR)
                nc.scalar.activation(t["expST"][:, m8, :], sp, AF.Exp,
                                     scale=EXPS_S)

            def stage_rsum(b):
                t = T[b]
                rp = ps()
                for nh in range(2):
                    for i in range(4):
                        nc.tensor.matmul(
                            nb(rp, nh), ones2,
                            t["expST"][:, 2 * i:2 * i + 2,
                                       nh * 512:(nh + 1) * 512],
                            start=(i == 0), stop=(i == 3), perf_mode=DR)
                nc.vector.reciprocal_approx_fast(out=t["rinv"], in_=rp)

            def av_half(b, mc):
                t = T[b]
                ap2 = ps()
                for i in range(4):
                    for nh in range(2):
                        nc.tensor.matmul(
                            nb(ap2, nh),
                            t["v_nat"][:, 2 * i:2 * i + 2,
                                       mc * 128:(mc + 1) * 128],
                            t["expST"][:, 2 * i:2 * i + 2,
                                       nh * 512:(nh + 1) * 512],
                            start=(i == 0), stop=(i == 3), perf_mode=DR)
                nc.vector.tensor_tensor(t["tmp"][:, mc, :], ap2,
                                        t["rinv"], op=OP.mult)

            def c_qc(b, nh):
                t = T[b]
                qp = ps()
                for mc in range(2):
                    nc.tensor.matmul(
                        qp[:, mc * 512:(mc + 1) * 512],
                        wsl("wq_c")[:, :, mc, :],
                        t["hnmm"][:, :, nh * 512:(nh + 1) * 512],
                        start=True, stop=False, perf_mode=DR,
                        skip_group_check=True)
                    nc.tensor.matmul(
                        qp[:, mc * 512:(mc + 1) * 512],
                        wsl("wq_c")[:, :, mc, :],
                        t["tmp"][:, :, nh * 512:(nh + 1) * 512],
                        start=False, stop=True, perf_mode=DR,
                        skip_group_check=True)
                for mc in range(2):
                    if b == 0:
                        nc.vector.tensor_scalar(
                            out=t["qcT"][:, mc, nh * 512:(nh + 1) * 512],
                            in0=qp[:, mc * 512:(mc + 1) * 512],
                            scalar1=QS / WS,
                            scalar2=cols["bq_c2"][:, mc:mc + 1],
                            op0=OP.mult, op1=OP.add)
                    else:
                        nc.scalar.activation(
                            out=t["qcT"][:, mc, nh * 512:(nh + 1) * 512],
                            in_=qp[:, mc * 512:(mc + 1) * 512],
                            func=AF.Identity,
                            bias=cols["bq_c2"][:, mc:mc + 1],
                            scale=QS / WS)

            def c_sc(b, nh):
                t = T[b]
                scp = ps5()
                nc.tensor.matmul(
                    scp[0:SP, 0:512], t["kcT"][:],
                    t["qcT"][:, :, nh * 512:(nh + 1) * 512],
                    start=True, stop=True, perf_mode=DR)
                nc.scalar.activation(
                    t["expScT"][:, nh * 512:(nh + 1) * 512],
                    scp[0:S, 0:512], AF.Exp, scale=EXPS_C)

            def c_fin_a(b, nh):
                # cross rowsum -> rcinv (DVE) -> escn = E*rcinv (GPSIMD)
                t = T[b]
                esl = t["expScT"][:, nh * 512:(nh + 1) * 512]
                crp = ps5()
                nc.tensor.matmul(crp[:, 0:512], onesc, esl,
                                 start=True, stop=True)
                rsl = t["rcinv"][:, nh * 512:(nh + 1) * 512]
                nc.vector.reciprocal_approx_fast(out=rsl, in_=crp[:, 0:512])
                nc.gpsimd.tensor_tensor(
                    t["escn"][:, nh * 512:(nh + 1) * 512], esl,
                    t["rcinv"][0:S, nh * 512:(nh + 1) * 512], op=OP.mult)

            def c_fin_b(b, nh):
                # attnV over projected values -> y = psum/HCS + x -> DMA out
                t = T[b]
                enl = t["escn"][:, nh * 512:(nh + 1) * 512]
                hcp = ps()
                for mc in range(2):
                    nc.tensor.matmul(
                        hcp[:, mc * 512:(mc + 1) * 512],
                        t["vcp"][:, mc * 128:(mc + 1) * 128], enl,
                        start=True, stop=False, skip_group_check=True)
                    nc.tensor.matmul(
                        hcp[:, mc * 512:(mc + 1) * 512], ident,
                        t["xT"][:, mc, nh * 512:(nh + 1) * 512],
                        start=False, stop=True, skip_group_check=True)
                nc.scalar.mul(
                    t["y_sb"][:, :, nh * 512:(nh + 1) * 512],
                    hcp[:].rearrange("p (m n) -> p m n", n=512), 1.0 / HCS)
                eng = nc.sync if nh == 0 else nc.scalar
                eng.dma_start(
                    out=y_d[b][:, nh],
                    in_=t["y_sb"][:, :, nh * 512:(nh + 1) * 512])

            # ================= schedule =================
            gn_stats(0)
            gn_group(0)
            gn_bcast(0)
            gn_affine(0)
            stageB_mm(0)
            stageB_mm(1)
            gv_g(0)
            stageB_evac(0)
            stageB_evac(1)
            gn_stats(1)
            vcp_mm(0)
            vcp_mm(1)
            gn_group(1)
            gv_v(0)
            spexp_one(0, 0)
            gn_bcast(1)
            spexp_one(0, 1)
            gn_affine(1)
            spexp_one(0, 2)
            spexp_one(0, 3)
            gv_g(1)
            for m8 in range(4, 8):
                spexp_one(0, m8)
            gv_v(1)
            for m8 in range(3):
                spexp_one(1, m8)
            stage_rsum(0)
            spexp_one(1, 3)
            av_half(0, 0)
            spexp_one(1, 4)
            spexp_one(1, 5)
            av_half(0, 1)
            spexp_one(1, 6)
            spexp_one(1, 7)
            c_qc(0, 0)
            c_qc(0, 1)
            c_sc(0, 0)
            c_sc(0, 1)
            stage_rsum(1)
            av_half(1, 0)
            av_half(1, 1)
            c_qc(1, 0)
            c_qc(1, 1)
            c_fin_a(0, 0)
            c_fin_a(0, 1)
            c_sc(1, 0)
            c_sc(1, 1)
            c_fin_b(0, 0)
            c_fin_b(0, 1)
            c_fin_a(1, 0)
            c_fin_b(1, 0)
            c_fin_a(1, 1)
            c_fin_b(1, 1)

    nc.finalize()
    return nc


def host_inputs(inputs):
    import ml_dtypes
    bf16 = ml_dtypes.bfloat16
    fp8 = ml_dtypes.float8_e4m3
    f = lambda a: np.ascontiguousarray(np.asarray(a, dtype=np.float32))
    # x: [B, C, HW] -> [B, 128(p), 2(a), HW] with c = a*128 + p
    x = f(inputs["x"]).reshape(B, 2, 128, HW).transpose(0, 2, 1, 3)
    x = np.ascontiguousarray(x).astype(bf16)
    # cemb: [B, S, CD] -> [B, 128(p), 4(k), SP] with cd = k*128 + p
    cembT = np.zeros((B, 128, 4, SP), np.float32)
    cembT[:, :, :, :S] = f(inputs["cemb"]).transpose(0, 2, 1).reshape(
        B, 4, 128, S).transpose(0, 2, 1, 3)
    cembT = cembT.reshape(B, 128, 4 * SP).astype(fp8)
    gsel = np.zeros((128, 16), np.float32)
    gsel[np.arange(128), np.arange(128) // 8] = 1.0
    wq_s, wk_s = f(inputs["wq_s"]), f(inputs["wk_s"])
    wmats = {
        "m2": GMS * (wq_s.T @ wk_s),  # already [kin, kout] layout
        "wv_s": WS * f(inputs["wv_s"]).T,
        "wq_c": WS * f(inputs["wq_c"]).T,
        "w_proj": WS * f(inputs["w_proj"]).T,
        "wk_c": WS * f(inputs["wk_c"]).T,
        "wv_c": WS * f(inputs["wv_c"]).T,
    }
    # pack: wall [128(p), 16(k), 2(m), 128(c)]; w row index kin = k*128 + p
    wall = np.zeros((128, 16, 2, 128), np.float32)
    for name, (k0, kch) in WPACK.items():
        w = wmats[name]  # [kin, 256]
        wall[:, k0:k0 + kch] = w.reshape(kch, 128, 2, 128).transpose(
            1, 0, 2, 3)
    colv = {
        "gn_gamma": f(inputs["gn_gamma"]),
        "gn_beta": f(inputs["gn_beta"]),
        "bg2": GQS * (f(inputs["bq_s"]) @ wk_s),
        "bq_c2": QS * (f(inputs["bq_c"])
                       + f(inputs["bv_s"]) @ f(inputs["wq_c"]).T),
    }
    # cols [128(p), 2*i + a] with c = a*128 + p
    cols = np.zeros((128, 2 * len(CPACK)), np.float32)
    for i, name in enumerate(CPACK):
        cols[:, 2 * i:2 * i + 2] = colv[name].reshape(2, 128).T
    brow = np.stack([
        VS2 * WS * f(inputs["b_proj"]),
        WS * f(inputs["bk_c"]),
        WS * f(inputs["bv_c"]),
    ]).astype(bf16)
    shared = {
        "ident": np.ascontiguousarray(HCS * np.eye(128, dtype=np.float32)
                                      ).astype(bf16),
        "wall": np.ascontiguousarray(wall).astype(fp8),
        "cols": cols,
        "brow": np.ascontiguousarray(brow),
        "gsel": gsel,
        "gbc": np.ascontiguousarray(gsel.T),
    }
    return [
        {"xbf": x[i * BPC:(i + 1) * BPC],
         "cembT": cembT[i * BPC:(i + 1) * BPC], **shared}
        for i in range(NCORES)
    ]


def kernel(**inputs):
    global LAST_RESULT
    from concourse.bass_utils import run_bass_kernel_spmd

    if "nc" not in _CACHE:
        _CACHE["nc"] = _build_nc()
    nc = _CACHE["nc"]

    in_maps = host_inputs(inputs)
    res = run_bass_kernel_spmd(nc, in_maps, list(range(NCORES)),
                               trace=bool(os.environ.get("BASS_TRACE")))
    LAST_RESULT = res
    # y [BPC, 128(p), 2(nh), 2(a), 512] -> [BPC, C = a*128+p, HW = nh*512+n]
    y = np.concatenate([res.results[i]["y"] for i in range(NCORES)], axis=0)
    y = y.transpose(0, 3, 1, 2, 4).reshape(B, C, HW)
    return y.reshape(B, C, H, W).astype(np.float32)


# revision 39
# speedup vs baseline: 1.0469x; 1.0469x over previous
"""AttnBlock (GroupNorm + self-attn + cross-attn + proj, residual) on 8 trn2 cores.

Sharding: data-parallel over batch B=16 -> 2 images per core; weights replicated.

v6: v5 + critical-path surgery.
 - head: x DMA gets the rings first (weights trail on the same queue);
   affine(0) and gT(0) evacuations split ACT/DVE so the serial
   GN -> affine -> g -> exp chain crosses engines in parallel halves.
 - kc/vc biases ride tiny PE bias-matmuls (brow stationary rows) so the
   stage-B evacuations collapse to one instruction each; kc+vcT share
   one PSUM bank per image.
 - gv(1) is emitted after image 0's exp stream starts; image 1's whole
   GN/gv pipeline hides under image 0's 8 exps.
 - tail: c_fin split into crp/rcinv/escn (escn on the idle GPSIMD) and
   attnV/y/dma stages, interleaved across the four (b, nh) pairs so the
   four cross-attn chains pipeline instead of running serially.
 - PSUM: 3x2-bank pool for the wide matmuls + 2x1-bank pool (warmup,
   groupnorm selector/broadcast, kc/vc/vcp, sc, crossrowsum).

Scale ledger (host WS=16 on true weights):
  M2' = GMS*(Wq^T Wk), GMS=128; gT = GQS*g + GQS*(bq Wk), GQS=8
  S^T psum = GQS*logits -> exp(scale=1/(16*GQS))
  v' = VSC*(hn Wv^T), VSC=2 = ones_self -> rinv = 1/(VSC*r), tmp = U/r fp8
  qc psum = WS*(Wqc(hn+tmp)); qcT = QS*qc + QS*(bqc + Wqc bv_s), QS=2
  kc psum = WS*(kc+bkc) (bias matmul) -> kcT = QS*(kc+bkc)
  vc psum = WS*(vc+bvc) -> vc_f8 = VS2*vc'; vcp psum = VS2*WS*(vc Wp^T)+bp
  vcp bf16 natural; onesc = 1/HCS (HCS=8)
  rcinv = HCS/rc; escn = E*rcinv (gpsimd); hc psum = HCS*out; y = psum/HCS + x
"""

import os

import numpy as np

B, C, H, W, S, CD = 16, 256, 32, 32, 77, 512
HW = H * W
SP = 80  # S padded to a 16B-aligned stride for DoubleRow APs
GROUPS = 32
GS = C // GROUPS
EPS = 1e-5
NCORES = 8
BPC = B // NCORES

WS = 16.0          # host-side weight scale (fp8 subnormal avoidance)
QS = 2.0           # qc/kc storage scale
GMS = 128.0        # host scale on M2 = Wq^T Wk
GQS = 8.0          # gT storage scale
EXPS_S = 1.0 / (16.0 * GQS)    # self exp scale
EXPS_C = 1.0 / (16.0 * QS * QS)  # cross exp scale
VSC = 2.0          # v storage scale == ones_self value
VS2 = 4.0          # vc fp8 storage scale
HCS = 8.0          # ones_cross = 1/HCS; final evac scale 1/HCS

# packed weight layout: name -> (k0, kch) into wall [128, 16, 2, 128]
WPACK = {"m2": (0, 2), "wv_s": (2, 2), "wq_c": (4, 2), "w_proj": (6, 2),
         "wk_c": (8, 4), "wv_c": (12, 4)}
CPACK = ["gn_gamma", "gn_beta", "bg2", "bq_c2"]

_CACHE = {}
LAST_RESULT = None  # test harness reads exec_time_ns off this


def _build_nc():
    import concourse.bacc as bacc
    import concourse.bass as bass
    import concourse.tile as tile
    from concourse import mybir

    f32 = mybir.dt.float32
    bf16 = mybir.dt.bfloat16
    fp8 = mybir.dt.float8e4
    AF = mybir.ActivationFunctionType
    OP = mybir.AluOpType
    DR = mybir.MatmulPerfMode.DoubleRow

    nc = bacc.Bacc("TRN2", target_bir_lowering=False, debug=False)

    x_d = nc.dram_tensor("xbf", [BPC, 128, 2, HW], bf16, kind="ExternalInput")
    cembT_d = nc.dram_tensor("cembT", [BPC, 128, 4 * SP], fp8,
                             kind="ExternalInput")
    wall_d = nc.dram_tensor("wall", [128, 16, 2, 128], fp8,
                            kind="ExternalInput")
    cols_d = nc.dram_tensor("cols", [128, 2 * len(CPACK)], f32,
                            kind="ExternalInput")
    brow_d = nc.dram_tensor("brow", [3, C], bf16, kind="ExternalInput")
    ident_d = nc.dram_tensor("ident", [128, 128], bf16, kind="ExternalInput")
    gsel_d = nc.dram_tensor("gsel", [128, 16], f32, kind="ExternalInput")
    gbc_d = nc.dram_tensor("gbc", [16, 128], f32, kind="ExternalInput")
    y_d = nc.dram_tensor("y", [BPC, 128, 2, 2, 512], bf16,
                         kind="ExternalOutput")

    with tile.TileContext(nc) as tc:
        with (
            tc.tile_pool(name="const", bufs=1) as const,
            tc.tile_pool(name="work", bufs=2) as work,
            tc.tile_pool(name="psp", bufs=3, space="PSUM") as psp,
            tc.tile_pool(name="pss", bufs=2, space="PSUM") as pss,
        ):
            # ---- constants (no DMA) ----
            ones2 = const.tile([128, 2, 128], fp8)
            nc.vector.memset(ones2, VSC)
            onesc = const.tile([S, 128], bf16)
            nc.vector.memset(onesc, 1.0 / HCS)
            ones_row = const.tile([1, 512], bf16)
            nc.vector.memset(ones_row, 1.0)
            # touch Exp once so its ACT table load overlaps the weight DMAs
            warm = const.tile([128, 1], f32)
            nc.vector.memset(warm, 0.0)
            nc.scalar.activation(warm, warm, AF.Exp)
            dummy_mov = const.tile([128, 2, 512], fp8)
            nc.vector.memset(dummy_mov, 1.0)

            ps = lambda: psp.tile([128, HW], f32, tag="ps", name="ps")
            ps5 = lambda: pss.tile([128, 512], f32, tag="sm", name="sm")

            def warm_burst(n):
                wup = ps5()
                for i in range(n):
                    nc.tensor.matmul(wup, ones2, dummy_mov,
                                     start=(i == 0), stop=(i == n - 1),
                                     perf_mode=DR)

            warm_burst(10)

            # ---- input DMAs ----
            # sync queue: x first (GN head is the critical path), then the
            # packed weights; scalar queue: selectors + cemb + bias rows
            xTs, cembTs = [], []
            for b in range(BPC):
                xT = work.tile([128, 2, HW], bf16, tag="xT")
                for a in range(2):
                    nc.sync.dma_start(out=xT[:, a, :], in_=x_d[b][:, a, :])
                xTs.append(xT)
            wall = const.tile([128, 16, 2, 128], fp8)
            nc.sync.dma_start(out=wall, in_=wall_d[:])
            gsel = const.tile([128, 16], f32)
            nc.scalar.dma_start(out=gsel, in_=gsel_d[:])
            gbc = const.tile([16, 128], f32)
            nc.scalar.dma_start(out=gbc, in_=gbc_d[:])
            for b in range(BPC):
                cembT = work.tile([128, 4, SP], fp8, tag="cembT")
                nc.scalar.dma_start(
                    out=cembT,
                    in_=cembT_d[b].rearrange("p (k s) -> p k s", s=SP))
                cembTs.append(cembT)
            ident = const.tile([128, 128], bf16)
            nc.scalar.dma_start(out=ident, in_=ident_d[:])
            brows = []
            for i in range(3):
                r = const.tile([1, C], bf16, tag=f"brow{i}")
                nc.scalar.dma_start(out=r, in_=brow_d[i:i + 1, :])
                brows.append(r)
            # gpsimd queue: packed bias/affine columns
            cols_all = const.tile([128, 2 * len(CPACK)], f32)
            nc.gpsimd.dma_start(out=cols_all, in_=cols_d[:])
            cols = {name: cols_all[:, 2 * i:2 * i + 2]
                    for i, name in enumerate(CPACK)}

            def wsl(name):
                k0, kch = WPACK[name]
                return wall[:, k0:k0 + kch, :, :]

            wvs_flat = wsl("wv_s").rearrange("p k m c -> p k (m c)")
            wproj_flat = wsl("w_proj").rearrange("p k m c -> p k (m c)")

            nb = lambda ap, nh: ap[:, nh * 512:(nh + 1) * 512]

            # per-image tiles
            T = [dict(xT=xTs[b], cembT=cembTs[b]) for b in range(BPC)]
            for b in range(BPC):
                t = T[b]
                for key, shape, dt_ in [
                    ("kcT", [128, 2, SP], fp8),
                    ("vc_f8", [128, 2, SP], fp8),
                    ("vcp", [S, C], bf16),
                    ("stats6", [128, 2, 2, 6], f32),
                    ("qsum", [128, 2, 2], f32),
                    ("m2sum", [128, 2, 2], f32),
                    ("msq_e", [128, 2, 2], f32),
                    ("musq", [128, 2, 2], f32),
                    ("spack", [128, 3, 2, 1], f32),
                    ("tm", [16, 2], f32),
                    ("ex2", [16, 2], f32),
                    ("msq", [16, 2], f32),
                    ("varv", [16, 2], f32),
                    ("ya", [16, 2], f32),
                    ("yb", [16, 2], f32),
                    ("y2", [16, 2], f32),
                    ("mrp", [16, 4], f32),
                    ("Acol", [128, 2], f32),
                    ("Bcol", [128, 2], f32),
                    ("t1", [128, 2], f32),
                    ("hnmm", [128, 2, HW], fp8),
                    ("gT", [128, 2, HW], fp8),
                    ("v_nat", [128, 8, C], fp8),
                    ("expST", [128, 8, HW], fp8),
                    ("rinv", [128, HW], f32),
                    ("tmp", [128, 2, HW], fp8),
                    ("qcT", [128, 2, HW], fp8),
                    ("expScT", [S, HW], bf16),
                    ("escn", [S, HW], bf16),
                    ("rcinv", [128, HW], f32),
                    ("y_sb", [128, 2, HW], bf16),
                ]:
                    t[key] = work.tile(shape, dt_, tag=key, name=key)

            def gn_stats(b):
                t = T[b]
                AX = mybir.AxisListType
                for a in range(2):
                    for ch in range(2):
                        nc.vector.bn_stats(
                            t["stats6"][:, a, ch, :],
                            t["xT"][:, a, ch * 512:(ch + 1) * 512])
                s6 = t["stats6"]
                m_e, m_o = s6[:, :, :, 1:2], s6[:, :, :, 4:5]
                v_e, v_o = s6[:, :, :, 2:3], s6[:, :, :, 5:6]
                nc.vector.tensor_add(t["qsum"], m_e, m_o)
                nc.vector.tensor_add(t["m2sum"], v_e, v_o)
                nc.vector.tensor_mul(t["msq_e"], m_e, m_e)
                nc.vector.tensor_mul(t["musq"], m_o, m_o)
                nc.vector.tensor_add(t["musq"], t["musq"], t["msq_e"])
                nc.vector.reduce_sum(out=t["spack"][:, 0, :, :],
                                     in_=t["qsum"], axis=AX.X)
                nc.vector.reduce_sum(out=t["spack"][:, 1, :, :],
                                     in_=t["m2sum"], axis=AX.X)
                nc.vector.reduce_sum(out=t["spack"][:, 2, :, :],
                                     in_=t["musq"], axis=AX.X)

            def gn_group(b):
                t = T[b]
                gps = ps5()
                nc.tensor.matmul(gps[0:16, 0:6], gsel, t["spack"],
                                 start=True, stop=True)
                nc.vector.tensor_scalar_mul(t["mrp"][:, 0:2], gps[0:16, 0:2],
                                            1.0 / 32.0)
                nc.vector.tensor_scalar_mul(t["tm"], gps[0:16, 2:4],
                                            1.0 / 8192.0)
                nc.vector.scalar_tensor_tensor(
                    out=t["ex2"], in0=gps[0:16, 4:6], scalar=1.0 / 32.0,
                    in1=t["tm"], op0=OP.mult, op1=OP.add)
                nc.vector.tensor_mul(t["msq"], t["mrp"][:, 0:2],
                                     t["mrp"][:, 0:2])
                nc.vector.tensor_sub(t["varv"], t["ex2"], t["msq"])
                nc.vector.tensor_scalar_add(t["varv"], t["varv"], EPS)
                nc.vector.reciprocal_approx_fast(out=t["ya"], in_=t["varv"])
                cur = t["ya"]
                for it in range(1):
                    nc.vector.tensor_mul(t["y2"], cur, cur)
                    nc.vector.tensor_mul(t["y2"], t["y2"], t["varv"])
                    nc.vector.tensor_scalar(out=t["y2"], in0=t["y2"],
                                            scalar1=-0.5, scalar2=1.5,
                                            op0=OP.mult, op1=OP.add)
                    nxt = t["yb"] if cur is t["ya"] else t["ya"]
                    nc.vector.tensor_mul(nxt, cur, t["y2"])
                    cur = nxt
                nc.vector.tensor_copy(t["mrp"][:, 2:4], cur)

            def gn_bcast(b):
                t = T[b]
                mps = ps5()
                nc.tensor.matmul(mps[0:128, 0:4], gbc, t["mrp"],
                                 start=True, stop=True)
                t["mps"] = mps

            def gn_affine(b):
                # image 0: a=0 on ACT, a=1 on DVE (parallel halves)
                t = T[b]
                mps = t["mps"]
                nc.vector.tensor_mul(t["Acol"], mps[0:128, 2:4],
                                     cols["gn_gamma"])
                nc.vector.tensor_mul(t["t1"], mps[0:128, 0:2], t["Acol"])
                nc.vector.tensor_sub(t["Bcol"], cols["gn_beta"], t["t1"])
                for a in range(2):
                    if b == 0 and a == 0:
                        nc.scalar.activation(
                            out=t["hnmm"][:, a, :], in_=t["xT"][:, a, :],
                            func=AF.Identity,
                            bias=t["Bcol"][:, a:a + 1],
                            scale=t["Acol"][:, a:a + 1])
                    else:
                        nc.vector.tensor_scalar(
                            out=t["hnmm"][:, a, :], in0=t["xT"][:, a, :],
                            scalar1=t["Acol"][:, a:a + 1],
                            scalar2=t["Bcol"][:, a:a + 1],
                            op0=OP.mult, op1=OP.add)

            def stageB_mm(b):
                # kc and vcT share one 1-bank psum; biases via tiny matmuls
                t = T[b]
                bg = ps5()
                t["bigB"] = bg
                for w, wname, brow_i in ((0, "wk_c", 1), (1, "wv_c", 2)):
                    for mc in range(2):
                        dst = bg[:, (2 * w + mc) * 128:(2 * w + mc) * 128 + SP]
                        nc.tensor.matmul(
                            dst, brows[brow_i][0:1,
                                               mc * 128:(mc + 1) * 128],
                            ones_row[0:1, 0:SP],
                            start=True, stop=False, skip_group_check=True)
                        for i in range(2):
                            nc.tensor.matmul(
                                dst, wsl(wname)[:, 2 * i:2 * i + 2, mc, :],
                                t["cembT"][:, 2 * i:2 * i + 2, :],
                                start=False, stop=(i == 1), perf_mode=DR,
                                skip_group_check=True)

            def stageB_evac(b):
                t = T[b]
                bg = t["bigB"]
                nc.vector.memset(t["kcT"][:, :, S:SP], 0.0)
                nc.vector.memset(t["vc_f8"][:, :, S:SP], 0.0)
                src = bg[:].rearrange("p (g s) -> p g s", s=128)
                nc.scalar.mul(t["kcT"][:, :, 0:S], src[:, 0:2, 0:S], QS / WS)
                nc.vector.tensor_scalar_mul(
                    t["vc_f8"][:, :, 0:S], src[:, 2:4, 0:S], VS2 / WS)

            def vcp_mm(b):
                # vcp = vc @ Wp^T + bp on [77, 256] (proj folded into values)
                t = T[b]
                vcp_ps = ps5()
                nc.tensor.matmul(
                    vcp_ps[0:SP, 0:C], ones_row[0:1, 0:SP], brows[0][0:1, :],
                    start=True, stop=False, skip_group_check=True)
                nc.tensor.matmul(
                    vcp_ps[0:SP, 0:C], t["vc_f8"][:, :, 0:SP], wproj_flat,
                    start=False, stop=True, perf_mode=DR,
                    skip_group_check=True)
                nc.vector.tensor_scalar_mul(
                    t["vcp"], vcp_ps[0:S, 0:C], 1.0 / (VS2 * WS))

            def gv_g(b):
                # g = hn M2 + bg; image 0 evacs split ACT/DVE
                t = T[b]
                for mc in range(2):
                    qp = ps()
                    for nh in range(2):
                        nc.tensor.matmul(
                            nb(qp, nh), wsl("m2")[:, :, mc, :],
                            t["hnmm"][:, :, nh * 512:(nh + 1) * 512],
                            start=True, stop=True, perf_mode=DR)
                    if b == 0 and mc == 0:
                        nc.scalar.activation(
                            out=t["gT"][:, mc, :], in_=qp, func=AF.Identity,
                            bias=cols["bg2"][:, mc:mc + 1], scale=GQS / GMS)
                    else:
                        nc.vector.tensor_scalar(
                            out=t["gT"][:, mc, :], in0=qp, scalar1=GQS / GMS,
                            scalar2=cols["bg2"][:, mc:mc + 1],
                            op0=OP.mult, op1=OP.add)

            def gv_v(b):
                t = T[b]
                for half in range(2):
                    vp = ps()
                    for j in range(4):
                        m8 = 4 * half + j
                        nc.tensor.matmul(
                            vp[:, j * 256:(j + 1) * 256],
                            t["hnmm"][:, :, m8 * 128:(m8 + 1) * 128],
                            wvs_flat,
                            start=True, stop=True, perf_mode=DR)
                    vdst = t["v_nat"][:, 4 * half:4 * half + 4, :]
                    vsrc = vp[:].rearrange("p (j c) -> p j c", c=256)
                    nc.vector.tensor_scalar_mul(vdst, vsrc, VSC / WS)

            def spexp_one(b, m8):
                t = T[b]
                sp = ps()
                for nh in range(2):
                    nc.tensor.matmul(
                        nb(sp, nh), t["hnmm"][:, :, m8 * 128:(m8 + 1) * 128],
                        t["gT"][:, :, nh * 512:(nh + 1) * 512],
                        start=True, stop=True, perf_mode=DR)
                nc.scalar.activation(t["expST"][:, m8, :], sp, AF.Exp,
                                     scale=EXPS_S)

            def stage_rsum(b):
                t = T[b]
                rp = ps()
                for nh in range(2):
                    for i in range(4):
                        nc.tensor.matmul(
                            nb(rp, nh), ones2,
                            t["expST"][:, 2 * i:2 * i + 2,
                                       nh * 512:(nh + 1) * 512],
                            start=(i == 0), stop=(i == 3), perf_mode=DR)
                nc.vector.reciprocal_approx_fast(out=t["rinv"], in_=rp)

            def av_half(b, mc):
                t = T[b]
                ap2 = ps()
                for i in range(4):
                    for nh in range(2):
                        nc.tensor.matmul(
                            nb(ap2, nh),
                            t["v_nat"][:, 2 * i:2 * i + 2,
                                       mc * 128:(mc + 1) * 128],
                            t["expST"][:, 2 * i:2 * i + 2,
                                       nh * 512:(nh + 1) * 512],
                            start=(i == 0), stop=(i == 3), perf_mode=DR)
                nc.vector.tensor_tensor(t["tmp"][:, mc, :], ap2,
                                        t["rinv"], op=OP.mult)

            def c_qc(b, nh):
                t = T[b]
                qp = ps()
                for mc in range(2):
                    nc.tensor.matmul(
                        qp[:, mc * 512:(mc + 1) * 512],
                        wsl("wq_c")[:, :, mc, :],
                        t["hnmm"][:, :, nh * 512:(nh + 1) * 512],
                        start=True, stop=False, perf_mode=DR,
                        skip_group_check=True)
                    nc.tensor.matmul(
                        qp[:, mc * 512:(mc + 1) * 512],
                        wsl("wq_c")[:, :, mc, :],
                        t["tmp"][:, :, nh * 512:(nh + 1) * 512],
                        start=False, stop=True, perf_mode=DR,
                        skip_group_check=True)
                for mc in range(2):
                    if b == 0:
                        nc.vector.tensor_scalar(
                            out=t["qcT"][:, mc, nh * 512:(nh + 1) * 512],
                            in0=qp[:, mc * 512:(mc + 1) * 512],
                            scalar1=QS / WS,
                            scalar2=cols["bq_c2"][:, mc:mc + 1],
                            op0=OP.mult, op1=OP.add)
                    else:
                        nc.scalar.activation(
                            out=t["qcT"][:, mc, nh * 512:(nh + 1) * 512],
                            in_=qp[:, mc * 512:(mc + 1) * 512],
                            func=AF.Identity,
                            bias=cols["bq_c2"][:, mc:mc + 1],
                            scale=QS / WS)

            def c_sc(b, nh):
                t = T[b]
                scp = ps5()
                nc.tensor.matmul(
                    scp[0:SP, 0:512], t["kcT"][:],
                    t["qcT"][:, :, nh * 512:(nh + 1) * 512],
                    start=True, stop=True, perf_mode=DR)
                nc.scalar.activation(
                    t["expScT"][:, nh * 512:(nh + 1) * 512],
                    scp[0:S, 0:512], AF.Exp, scale=EXPS_C)

            def c_fin_a(b, nh):
                # cross rowsum -> rcinv (DVE) -> escn = E*rcinv (GPSIMD)
                t = T[b]
                esl = t["expScT"][:, nh * 512:(nh + 1) * 512]
                crp = ps5()
                nc.tensor.matmul(crp[:, 0:512], onesc, esl,
                                 start=True, stop=True)
                rsl = t["rcinv"][:, nh * 512:(nh + 1) * 512]
                nc.vector.reciprocal_approx_fast(out=rsl, in_=crp[:, 0:512])
                nc.gpsimd.tensor_tensor(
                    t["escn"][:, nh * 512:(nh + 1) * 512], esl,
                    t["rcinv"][0:S, nh * 512:(nh + 1) * 512], op=OP.mult)

            def c_fin_b(b, nh):
                # attnV over projected values -> y = psum/HCS + x -> DMA out
                t = T[b]
                enl = t["escn"][:, nh * 512:(nh + 1) * 512]
                hcp = ps()
                for mc in range(2):
                    nc.tensor.matmul(
                        hcp[:, mc * 512:(mc + 1) * 512],
                        t["vcp"][:, mc * 128:(mc + 1) * 128], enl,
                        start=True, stop=False, skip_group_check=True)
                    nc.tensor.matmul(
                        hcp[:, mc * 512:(mc + 1) * 512], ident,
                        t["xT"][:, mc, nh * 512:(nh + 1) * 512],
                        start=False, stop=True, skip_group_check=True)
                nc.scalar.mul(
                    t["y_sb"][:, :, nh * 512:(nh + 1) * 512],
                    hcp[:].rearrange("p (m n) -> p m n", n=512), 1.0 / HCS)
                eng = nc.sync if nh == 0 else nc.scalar
                eng.dma_start(
                    out=y_d[b][:, nh],
                    in_=t["y_sb"][:, :, nh * 512:(nh + 1) * 512])

            # ================= schedule =================
            gn_stats(0)
            gn_group(0)
            gn_bcast(0)
            gn_affine(0)
            stageB_mm(0)
            stageB_mm(1)
            gv_g(0)
            stageB_evac(0)
            stageB_evac(1)
            gn_stats(1)
            vcp_mm(0)
            vcp_mm(1)
            gv_v(0)
            for m8 in range(4):
                spexp_one(0, m8)
            gn_group(1)
            for m8 in range(4, 8):
                spexp_one(0, m8)
            gn_bcast(1)
            gn_affine(1)
            gv_g(1)
            gv_v(1)
            for m8 in range(3):
                spexp_one(1, m8)
            stage_rsum(0)
            spexp_one(1, 3)
            av_half(0, 0)
            spexp_one(1, 4)
            spexp_one(1, 5)
            av_half(0, 1)
            spexp_one(1, 6)
            spexp_one(1, 7)
            c_qc(0, 0)
            c_qc(0, 1)
            c_sc(0, 0)
            c_sc(0, 1)
            stage_rsum(1)
            av_half(1, 0)
            av_half(1, 1)
            c_qc(1, 0)
            c_qc(1, 1)
            c_fin_a(0, 0)
            c_fin_a(0, 1)
            c_sc(1, 0)
            c_sc(1, 1)
            c_fin_b(0, 0)
            c_fin_b(0, 1)
            c_fin_a(1, 0)
            c_fin_b(1, 0)
            c_fin_a(1, 1)
            c_fin_b(1, 1)

    nc.finalize()
    return nc


def host_inputs(inputs):
    import ml_dtypes
    bf16 = ml_dtypes.bfloat16
    fp8 = ml_dtypes.float8_e4m3
    f = lambda a: np.ascontiguousarray(np.asarray(a, dtype=np.float32))
    # x: [B, C, HW] -> [B, 128(p), 2(a), HW] with c = a*128 + p
    x = f(inputs["x"]).reshape(B, 2, 128, HW).transpose(0, 2, 1, 3)
    x = np.ascontiguousarray(x).astype(bf16)
    # cemb: [B, S, CD] -> [B, 128(p), 4(k), SP] with cd = k*128 + p
    cembT = np.zeros((B, 128, 4, SP), np.float32)
    cembT[:, :, :, :S] = f(inputs["cemb"]).transpose(0, 2, 1).reshape(
        B, 4, 128, S).transpose(0, 2, 1, 3)
    cembT = cembT.reshape(B, 128, 4 * SP).astype(fp8)
    gsel = np.zeros((128, 16), np.float32)
    gsel[np.arange(128), np.arange(128) // 8] = 1.0
    wq_s, wk_s = f(inputs["wq_s"]), f(inputs["wk_s"])
    wmats = {
        "m2": GMS * (wq_s.T @ wk_s),  # already [kin, kout] layout
        "wv_s": WS * f(inputs["wv_s"]).T,
        "wq_c": WS * f(inputs["wq_c"]).T,
        "w_proj": WS * f(inputs["w_proj"]).T,
        "wk_c": WS * f(inputs["wk_c"]).T,
        "wv_c": WS * f(inputs["wv_c"]).T,
    }
    # pack: wall [128(p), 16(k), 2(m), 128(c)]; w row index kin = k*128 + p
    wall = np.zeros((128, 16, 2, 128), np.float32)
    for name, (k0, kch) in WPACK.items():
        w = wmats[name]  # [kin, 256]
        wall[:, k0:k0 + kch] = w.reshape(kch, 128, 2, 128).transpose(
            1, 0, 2, 3)
    colv = {
        "gn_gamma": f(inputs["gn_gamma"]),
        "gn_beta": f(inputs["gn_beta"]),
        "bg2": GQS * (f(inputs["bq_s"]) @ wk_s),
        "bq_c2": QS * (f(inputs["bq_c"])
                       + f(inputs["bv_s"]) @ f(inputs["wq_c"]).T),
    }
    # cols [128(p), 2*i + a] with c = a*128 + p
    cols = np.zeros((128, 2 * len(CPACK)), np.float32)
    for i, name in enumerate(CPACK):
        cols[:, 2 * i:2 * i + 2] = colv[name].reshape(2, 128).T
    brow = np.stack([
        VS2 * WS * f(inputs["b_proj"]),
        WS * f(inputs["bk_c"]),
        WS * f(inputs["bv_c"]),
    ]).astype(bf16)
    shared = {
        "ident": np.ascontiguousarray(HCS * np.eye(128, dtype=np.float32)
                                      ).astype(bf16),
        "wall": np.ascontiguousarray(wall).astype(fp8),
        "cols": cols,
        "brow": np.ascontiguousarray(brow),
        "gsel": gsel,
        "gbc": np.ascontiguousarray(gsel.T),
    }
    return [
        {"xbf": x[i * BPC:(i + 1) * BPC],
         "cembT": cembT[i * BPC:(i + 1) * BPC], **shared}
        for i in range(NCORES)
    ]


def kernel(**inputs):
    global LAST_RESULT
    from concourse.bass_utils import run_bass_kernel_spmd

    if "nc" not in _CACHE:
        _CACHE["nc"] = _build_nc()
    nc = _CACHE["nc"]

    in_maps = host_inputs(inputs)
    res = run_bass_kernel_spmd(nc, in_maps, list(range(NCORES)),
                               trace=bool(os.environ.get("BASS_TRACE")))
    LAST_RESULT = res
    # y [BPC, 128(p), 2(nh), 2(a), 512] -> [BPC, C = a*128+p, HW = nh*512+n]
    y = np.concatenate([res.results[i]["y"] for i in range(NCORES)], axis=0)
    y = y.transpose(0, 3, 1, 2, 4).reshape(B, C, HW)
    return y.reshape(B, C, H, W).astype(np.float32)
